# revision 11
# baseline (speedup 1.0000x reference)
"""Trainium2 Bass kernel for nn_ContinuousActor (GNN message passing actor MLP).

Strategy (pure data parallel over 8 cores, batch dim sharded):
  - Host repacks inputs feature-major: XT[74, B] = [obs.T; ag.T; g.T; ones].
    The ones row folds the (per-pair) phi1 bias into the matmul.
  - The per-pair input permutation/concat/one-hot of the reference is folded
    into 6 per-pair effective weight matrices W1e[p] of shape [74, 256]
    (host-side rearrangement of phi_w1 rows; one-hot rows fold into the bias).
  - On device everything is feature-major [features, batch]: per 512-col batch
    tile, 6x (phi1 matmul -> relu -> phi2 matmul -> relu+bias) then sum-pool,
    rho MLP, and the mean/logstd heads (with clip for logstd).
  - Matmuls run as float32r (full fp32 storage, fast PE path).
"""

import numpy as np
from contextlib import ExitStack

import concourse.bass as bass
import concourse.mybir as mybir
import concourse.tile as tile
from concourse import bacc
from concourse.bass_utils import run_bass_kernel_spmd

F32 = mybir.dt.float32
F32R = mybir.dt.float32r
RELU = mybir.ActivationFunctionType.Relu

B_FULL = 65536
N_CORES = 8
BC = B_FULL // N_CORES  # 8192 batch rows per core
BT = 512                # batch tile (matmul free dim)
KX = 74                 # 55 obs + 9 ag + 9 g + 1 ones
NB_OBJ = 3
DIM_BODY = 10
DIM_OBJECT = 15
PERMS = [(0, 1), (0, 2), (1, 0), (1, 2), (2, 0), (2, 1)]
LOG_SIG_MIN, LOG_SIG_MAX = -20.0, 2.0

_CACHE = {}


def _pack_weights(phi_w1, phi_b1, phi_w2, phi_b2, rho_w1, rho_b1,
                  mean_w, mean_b, logstd_w, logstd_b):
    """Host-side weight repacking into device layouts (all float32)."""
    f = np.float32
    # phi1: per-pair effective weights [74, 6*256]; ones-row (73) carries bias.
    w1 = np.zeros((KX, 6 * 256), dtype=f)
    for p, (i, j) in enumerate(PERMS):
        Wp = w1[:, p * 256:(p + 1) * 256]
        Wp[0:10] = phi_w1[12:22]                      # obs body
        Wp[10 + 15 * i:25 + 15 * i] = phi_w1[25:40]   # obj i features
        Wp[10 + 15 * j:25 + 15 * j] = phi_w1[43:58]   # obj j features
        Wp[55 + 3 * i:58 + 3 * i] = phi_w1[0:3]       # ag_i
        Wp[55 + 3 * j:58 + 3 * j] = phi_w1[3:6]       # ag_j
        Wp[64 + 3 * i:67 + 3 * i] = phi_w1[6:9]       # g_i
        Wp[64 + 3 * j:67 + 3 * j] = phi_w1[9:12]      # g_j
        Wp[73] = phi_b1 + phi_w1[22 + i] + phi_w1[40 + j]  # bias + one-hots
    # phi2 / rho: [128, 4*128] with col block (2k+m) = W[k*128:(k+1)*128, m*128:(m+1)*128]
    def pack_256(w):
        out = np.empty((128, 512), dtype=f)
        for k in range(2):
            for m in range(2):
                out[:, (2 * k + m) * 128:(2 * k + m + 1) * 128] = \
                    w[k * 128:(k + 1) * 128, m * 128:(m + 1) * 128]
        return out
    w2 = pack_256(np.asarray(phi_w2, f))
    wr = pack_256(np.asarray(rho_w1, f))
    b2 = np.asarray(phi_b2, f).reshape(2, 128).T.copy()   # [128, 2], col m
    br = np.asarray(rho_b1, f).reshape(2, 128).T.copy()
    # heads: [128, 16], col block k*8 = Wh[k*128:(k+1)*128, :]
    wh_full = np.concatenate([np.asarray(mean_w, f), np.asarray(logstd_w, f)], axis=1)  # [256, 8]
    wh = np.concatenate([wh_full[0:128, :], wh_full[128:256, :]], axis=1)  # [128, 16]
    bh = np.concatenate([np.asarray(mean_b, f), np.asarray(logstd_b, f)]).reshape(1, 8)
    # per-partition clip bounds for the 8 head rows: mean rows unclipped
    big = np.float32(3.0e38)
    clipb = np.empty((8, 2), dtype=f)
    clipb[0:4, 0], clipb[4:8, 0] = big, LOG_SIG_MAX   # hi (min op)
    clipb[0:4, 1], clipb[4:8, 1] = -big, LOG_SIG_MIN  # lo (max op)
    ones = np.ones((1, BT), dtype=f)
    return dict(w1=w1, w2=w2, b2=b2, wr=wr, br=br, wh=wh, bh=bh, clipb=clipb,
                ones=ones)


def _pack_xt(obs, ag, g):
    B = obs.shape[0]
    xt = np.empty((KX, B), dtype=np.float32)
    xt[0:55] = obs.T
    xt[55:64] = ag.T
    xt[64:73] = g.T
    xt[73] = 1.0
    return xt


def _build_bass(bc, bt):
    """Build the per-core Bass program for a core batch of `bc` rows, tiled by `bt`."""
    nt = bc // bt
    nc = bacc.Bacc(trn_type="TRN2")

    xt_d = nc.dram_tensor("xt", [KX, bc], F32R, kind="ExternalInput")
    w1_d = nc.dram_tensor("w1", [KX, 6 * 256], F32R, kind="ExternalInput")
    w2_d = nc.dram_tensor("w2", [128, 512], F32R, kind="ExternalInput")
    b2_d = nc.dram_tensor("b2", [128, 2], F32, kind="ExternalInput")
    wr_d = nc.dram_tensor("wr", [128, 512], F32R, kind="ExternalInput")
    br_d = nc.dram_tensor("br", [128, 2], F32, kind="ExternalInput")
    wh_d = nc.dram_tensor("wh", [128, 16], F32R, kind="ExternalInput")
    bh_d = nc.dram_tensor("bh", [1, 8], F32R, kind="ExternalInput")
    clipb_d = nc.dram_tensor("clipb", [8, 2], F32, kind="ExternalInput")
    ones_d = nc.dram_tensor("ones", [1, bt], F32R, kind="ExternalInput")
    y_d = nc.dram_tensor("y", [8, bc], F32, kind="ExternalOutput")

    with ExitStack() as ctx:
        tc = ctx.enter_context(tile.TileContext(nc))
        consts = ctx.enter_context(tc.tile_pool(name="consts", bufs=1))
        sbp = ctx.enter_context(tc.tile_pool(name="sbp", bufs=3))
        psp = ctx.enter_context(tc.tile_pool(name="psp", bufs=2, space="PSUM"))

        w1sb = consts.tile([KX, 6 * 256], F32R)
        nc.sync.dma_start(out=w1sb, in_=w1_d[:, :])
        w2sb = consts.tile([128, 512], F32R)
        nc.sync.dma_start(out=w2sb, in_=w2_d[:, :])
        wrsb = consts.tile([128, 512], F32R)
        nc.sync.dma_start(out=wrsb, in_=wr_d[:, :])
        whsb = consts.tile([128, 16], F32R)
        nc.sync.dma_start(out=whsb, in_=wh_d[:, :])
        b2sb = consts.tile([128, 2], F32)
        nc.sync.dma_start(out=b2sb, in_=b2_d[:, :])
        brsb = consts.tile([128, 2], F32)
        nc.sync.dma_start(out=brsb, in_=br_d[:, :])
        bhsb = consts.tile([1, 8], F32R)
        nc.sync.dma_start(out=bhsb, in_=bh_d[:, :])
        clipsb = consts.tile([8, 2], F32)
        nc.sync.dma_start(out=clipsb, in_=clipb_d[:, :])
        ones_sb = consts.tile([1, bt], F32R)
        nc.sync.dma_start(out=ones_sb, in_=ones_d[:, :])

        for t in range(nt):
            s0 = t * bt
            xts = sbp.tile([KX, bt], F32R, tag="xts")
            nc.sync.dma_start(out=xts, in_=xt_d[:, s0:s0 + bt])

            pooled = sbp.tile([128, 2 * bt], F32R, tag="pooled")
            for p in range(6):
                ph1 = psp.tile([128, 2 * bt], F32, tag="ph1")
                for m in range(2):
                    nc.tensor.matmul(
                        ph1[:, m * bt:(m + 1) * bt],
                        w1sb[:, p * 256 + m * 128:p * 256 + (m + 1) * 128],
                        xts,
                        start=True, stop=True,
                    )
                h1 = sbp.tile([128, 2 * bt], F32R, tag="h1")
                nc.vector.tensor_scalar_max(h1, ph1, 0.0)  # relu (DVE)

                ph2 = psp.tile([128, 2 * bt], F32, tag="ph2")
                for m in range(2):
                    for k in range(2):
                        nc.tensor.matmul(
                            ph2[:, m * bt:(m + 1) * bt],
                            w2sb[:, (2 * k + m) * 128:(2 * k + m + 1) * 128],
                            h1[:, k * bt:(k + 1) * bt],
                            start=(k == 0), stop=(k == 1),
                        )
                if p == 0:
                    for m in range(2):
                        nc.scalar.activation(
                            pooled[:, m * bt:(m + 1) * bt],
                            ph2[:, m * bt:(m + 1) * bt],
                            RELU, bias=b2sb[:, m:m + 1],
                        )
                else:
                    tmp = sbp.tile([128, 2 * bt], F32R, tag=f"tmp{p % 2}")
                    for m in range(2):
                        nc.scalar.activation(
                            tmp[:, m * bt:(m + 1) * bt],
                            ph2[:, m * bt:(m + 1) * bt],
                            RELU, bias=b2sb[:, m:m + 1],
                        )
                    eng = nc.vector if p % 2 == 1 else nc.gpsimd
                    eng.tensor_add(pooled, pooled, tmp)

            # rho layer
            pr = psp.tile([128, 2 * bt], F32, tag="ph1")
            for m in range(2):
                for k in range(2):
                    nc.tensor.matmul(
                        pr[:, m * bt:(m + 1) * bt],
                        wrsb[:, (2 * k + m) * 128:(2 * k + m + 1) * 128],
                        pooled[:, k * bt:(k + 1) * bt],
                        start=(k == 0), stop=(k == 1),
                    )
            xs = sbp.tile([128, 2 * bt], F32R, tag="xs")
            for m in range(2):
                nc.scalar.activation(
                    xs[:, m * bt:(m + 1) * bt],
                    pr[:, m * bt:(m + 1) * bt],
                    RELU, bias=brsb[:, m:m + 1],
                )

            # heads: py = Wh.T @ xs + bh (bias via ones-row matmul)
            py = psp.tile([8, bt], F32, tag="ph2")
            for k in range(2):
                nc.tensor.matmul(
                    py, whsb[:, k * 8:(k + 1) * 8], xs[:, k * bt:(k + 1) * bt],
                    start=(k == 0), stop=False,
                )
            nc.tensor.matmul(py, bhsb, ones_sb, start=False, stop=True)

            ysb = sbp.tile([8, bt], F32, tag="ysb")
            nc.vector.tensor_scalar(
                ysb, py, clipsb[:, 0:1], clipsb[:, 1:2],
                op0=mybir.AluOpType.min, op1=mybir.AluOpType.max,
            )
            nc.sync.dma_start(out=y_d[:, s0:s0 + bt], in_=ysb)

    return nc


def _get_nc(bc, bt):
    key = (bc, bt)
    if key not in _CACHE:
        nc = _build_bass(bc, bt)
        nc.finalize()  # Bacc: run compile passes (wait-splitting, reg alloc)
        _CACHE[key] = nc
    return _CACHE[key]


def kernel(obs, ag, g, phi_w1, phi_b1, phi_w2, phi_b2,
           rho_w1, rho_b1, mean_w, mean_b, logstd_w, logstd_b):
    obs = np.asarray(obs, np.float32)
    ag = np.asarray(ag, np.float32)
    g = np.asarray(g, np.float32)
    B = obs.shape[0]
    assert B == B_FULL, f"kernel hardcoded for B={B_FULL}, got {B}"

    packed = _pack_weights(phi_w1, phi_b1, phi_w2, phi_b2, rho_w1, rho_b1,
                           mean_w, mean_b, logstd_w, logstd_b)
    xt = _pack_xt(obs, ag, g)

    nc = _get_nc(BC, BT)
    in_maps = []
    for c in range(N_CORES):
        m = dict(packed)
        m["xt"] = np.ascontiguousarray(xt[:, c * BC:(c + 1) * BC])
        in_maps.append(m)

    import os
    trace = bool(os.environ.get("KERNEL_TRACE"))
    res = run_bass_kernel_spmd(nc, in_maps, core_ids=list(range(N_CORES)),
                               trace=trace)
    global _last_results
    _last_results = res

    y = np.concatenate([res.results[c]["y"] for c in range(N_CORES)], axis=1)  # [8, B]
    out = np.ascontiguousarray(y.T)  # [B, 8]
    mean = out[:, 0:4].copy()
    logstd = out[:, 4:8].copy()
    return mean, logstd


_last_results = None


# revision 22
# speedup vs baseline: 1.0307x; 1.0307x over previous
"""Trainium2 Bass kernel for nn_ContinuousActor (GNN message passing actor MLP).

Strategy (pure data parallel over 8 cores, batch dim sharded):
  - Host repacks inputs feature-major: XT[74, B] = [obs.T; ag.T; g.T; ones].
    The ones row folds the (per-pair) phi1 bias into the matmul.
  - The per-pair input permutation/concat/one-hot of the reference is folded
    into 6 per-pair effective weight matrices W1e[p] of shape [74, 256]
    (host-side rearrangement of phi_w1 rows; one-hot rows fold into the bias).
  - On device everything is feature-major [features, batch]: per 512-col batch
    tile, 6x (phi1 matmul -> relu -> phi2 matmul -> relu+bias) then sum-pool,
    rho MLP, and the mean/logstd heads (with clip for logstd).
  - Matmuls run as float32r (full fp32 storage, fast PE path).
"""

import numpy as np
import ml_dtypes
from contextlib import ExitStack

import concourse.bass as bass
import concourse.mybir as mybir
import concourse.tile as tile
from concourse import bacc
from concourse.bass_utils import run_bass_kernel_spmd

F32 = mybir.dt.float32
F32R = mybir.dt.float32r
BF16 = mybir.dt.bfloat16
RELU = mybir.ActivationFunctionType.Relu

# matmul input dtype: BF16 (fast PE path w/ FWL) or F32R (accurate, ~2x slower)
DT_MM = BF16
DT_NP = ml_dtypes.bfloat16 if DT_MM == BF16 else np.float32

B_FULL = 65536
N_CORES = 8
BC = B_FULL // N_CORES  # 8192 batch rows per core
BT = 512                # batch tile (matmul free dim)
KX = 74                 # 55 obs + 9 ag + 9 g + 1 ones
NB_OBJ = 3
DIM_BODY = 10
DIM_OBJECT = 15
PERMS = [(0, 1), (0, 2), (1, 0), (1, 2), (2, 0), (2, 1)]
LOG_SIG_MIN, LOG_SIG_MAX = -20.0, 2.0

_CACHE = {}


def _pack_weights(phi_w1, phi_b1, phi_w2, phi_b2, rho_w1, rho_b1,
                  mean_w, mean_b, logstd_w, logstd_b):
    """Host-side weight repacking into device layouts (all float32)."""
    f = np.float32
    # phi1: per-pair effective weights [74, 6*256]; ones-row (73) carries bias.
    w1 = np.zeros((KX, 6 * 256), dtype=f)
    for p, (i, j) in enumerate(PERMS):
        Wp = w1[:, p * 256:(p + 1) * 256]
        Wp[0:10] = phi_w1[12:22]                      # obs body
        Wp[10 + 15 * i:25 + 15 * i] = phi_w1[25:40]   # obj i features
        Wp[10 + 15 * j:25 + 15 * j] = phi_w1[43:58]   # obj j features
        Wp[55 + 3 * i:58 + 3 * i] = phi_w1[0:3]       # ag_i
        Wp[55 + 3 * j:58 + 3 * j] = phi_w1[3:6]       # ag_j
        Wp[64 + 3 * i:67 + 3 * i] = phi_w1[6:9]       # g_i
        Wp[64 + 3 * j:67 + 3 * j] = phi_w1[9:12]      # g_j
        Wp[73] = phi_b1 + phi_w1[22 + i] + phi_w1[40 + j]  # bias + one-hots
    # phi2 / rho: [128, 4*128] with col block (2k+m) = W[k*128:(k+1)*128, m*128:(m+1)*128]
    def pack_256(w):
        out = np.empty((128, 512), dtype=f)
        for k in range(2):
            for m in range(2):
                out[:, (2 * k + m) * 128:(2 * k + m + 1) * 128] = \
                    w[k * 128:(k + 1) * 128, m * 128:(m + 1) * 128]
        return out
    w2 = pack_256(np.asarray(phi_w2, f))
    wr = pack_256(np.asarray(rho_w1, f))
    b2 = np.asarray(phi_b2, f).reshape(2, 128).T.copy()   # [128, 2], col m
    br = np.asarray(rho_b1, f).reshape(2, 128).T.copy()
    # heads: [128, 16], col block k*8 = Wh[k*128:(k+1)*128, :]
    wh_full = np.concatenate([np.asarray(mean_w, f), np.asarray(logstd_w, f)], axis=1)  # [256, 8]
    wh = np.concatenate([wh_full[0:128, :], wh_full[128:256, :]], axis=1)  # [128, 16]
    bh = np.concatenate([np.asarray(mean_b, f), np.asarray(logstd_b, f)]).reshape(1, 8)
    w1, w2, wr, wh, bh = (a.astype(DT_NP) for a in (w1, w2, wr, wh, bh))
    # per-partition clip bounds for the 8 head rows: mean rows unclipped
    big = np.float32(3.0e38)
    clipb = np.empty((8, 2), dtype=f)
    clipb[0:4, 0], clipb[4:8, 0] = big, LOG_SIG_MAX   # hi (min op)
    clipb[0:4, 1], clipb[4:8, 1] = -big, LOG_SIG_MIN  # lo (max op)
    ones = np.ones((1, BT), dtype=DT_NP)
    return dict(w1=w1, w2=w2, b2=b2, wr=wr, br=br, wh=wh, bh=bh, clipb=clipb,
                ones=ones)


def _pack_xt(obs, ag, g):
    B = obs.shape[0]
    xt = np.empty((KX, B), dtype=DT_NP)
    xt[0:55] = obs.T.astype(DT_NP)
    xt[55:64] = ag.T.astype(DT_NP)
    xt[64:73] = g.T.astype(DT_NP)
    xt[73] = np.asarray(1.0, DT_NP)
    return xt


def _build_bass(bc, bt):
    """Build the per-core Bass program for a core batch of `bc` rows, tiled by `bt`."""
    nt = bc // bt
    nc = bacc.Bacc(trn_type="TRN2")

    xt_d = nc.dram_tensor("xt", [KX, bc], DT_MM, kind="ExternalInput")
    w1_d = nc.dram_tensor("w1", [KX, 6 * 256], DT_MM, kind="ExternalInput")
    w2_d = nc.dram_tensor("w2", [128, 512], DT_MM, kind="ExternalInput")
    b2_d = nc.dram_tensor("b2", [128, 2], F32, kind="ExternalInput")
    wr_d = nc.dram_tensor("wr", [128, 512], DT_MM, kind="ExternalInput")
    br_d = nc.dram_tensor("br", [128, 2], F32, kind="ExternalInput")
    wh_d = nc.dram_tensor("wh", [128, 16], DT_MM, kind="ExternalInput")
    bh_d = nc.dram_tensor("bh", [1, 8], DT_MM, kind="ExternalInput")
    clipb_d = nc.dram_tensor("clipb", [8, 2], F32, kind="ExternalInput")
    ones_d = nc.dram_tensor("ones", [1, bt], DT_MM, kind="ExternalInput")
    y_d = nc.dram_tensor("y", [8, bc], F32, kind="ExternalOutput")

    with ExitStack() as ctx:
        tc = ctx.enter_context(tile.TileContext(nc))
        consts = ctx.enter_context(tc.tile_pool(name="consts", bufs=1))
        sbp = ctx.enter_context(tc.tile_pool(name="sbp", bufs=3))
        psp = ctx.enter_context(tc.tile_pool(name="psp", bufs=2, space="PSUM"))

        w1sb = consts.tile([KX, 6 * 256], DT_MM)
        nc.sync.dma_start(out=w1sb, in_=w1_d[:, :])
        w2sb = consts.tile([128, 512], DT_MM)
        nc.sync.dma_start(out=w2sb, in_=w2_d[:, :])
        wrsb = consts.tile([128, 512], DT_MM)
        nc.sync.dma_start(out=wrsb, in_=wr_d[:, :])
        whsb = consts.tile([128, 16], DT_MM)
        nc.sync.dma_start(out=whsb, in_=wh_d[:, :])
        b2sb = consts.tile([128, 2], F32)
        nc.sync.dma_start(out=b2sb, in_=b2_d[:, :])
        brsb = consts.tile([128, 2], F32)
        nc.sync.dma_start(out=brsb, in_=br_d[:, :])
        bhsb = consts.tile([1, 8], DT_MM)
        nc.sync.dma_start(out=bhsb, in_=bh_d[:, :])
        clipsb = consts.tile([8, 2], F32)
        nc.sync.dma_start(out=clipsb, in_=clipb_d[:, :])
        ones_sb = consts.tile([1, bt], DT_MM)
        nc.sync.dma_start(out=ones_sb, in_=ones_d[:, :])

        for t in range(nt):
            s0 = t * bt
            xts = sbp.tile([KX, bt], DT_MM, tag="xts")
            nc.sync.dma_start(out=xts, in_=xt_d[:, s0:s0 + bt])

            pooled = sbp.tile([128, 2 * bt], DT_MM, tag="pooled")
            for p in range(6):
                ph1 = psp.tile([128, 2 * bt], F32, tag="ph1")
                for m in range(2):
                    nc.tensor.matmul(
                        ph1[:, m * bt:(m + 1) * bt],
                        w1sb[:, p * 256 + m * 128:p * 256 + (m + 1) * 128],
                        xts,
                        start=True, stop=True,
                    )
                h1 = sbp.tile([128, 2 * bt], DT_MM, tag="h1")
                nc.vector.tensor_scalar_max(h1, ph1, 0.0)  # relu (DVE)

                ph2 = psp.tile([128, 2 * bt], F32, tag="ph2")
                for m in range(2):
                    for k in range(2):
                        nc.tensor.matmul(
                            ph2[:, m * bt:(m + 1) * bt],
                            w2sb[:, (2 * k + m) * 128:(2 * k + m + 1) * 128],
                            h1[:, k * bt:(k + 1) * bt],
                            start=(k == 0), stop=(k == 1),
                        )
                if p == 0:
                    for m in range(2):
                        nc.scalar.activation(
                            pooled[:, m * bt:(m + 1) * bt],
                            ph2[:, m * bt:(m + 1) * bt],
                            RELU, bias=b2sb[:, m:m + 1],
                        )
                else:
                    tmp = sbp.tile([128, 2 * bt], DT_MM, tag=f"tmp{p % 2}")
                    for m in range(2):
                        nc.scalar.activation(
                            tmp[:, m * bt:(m + 1) * bt],
                            ph2[:, m * bt:(m + 1) * bt],
                            RELU, bias=b2sb[:, m:m + 1],
                        )
                    nc.gpsimd.tensor_add(pooled, pooled, tmp)

            # rho layer
            pr = psp.tile([128, 2 * bt], F32, tag="ph1")
            for m in range(2):
                for k in range(2):
                    nc.tensor.matmul(
                        pr[:, m * bt:(m + 1) * bt],
                        wrsb[:, (2 * k + m) * 128:(2 * k + m + 1) * 128],
                        pooled[:, k * bt:(k + 1) * bt],
                        start=(k == 0), stop=(k == 1),
                    )
            xs = sbp.tile([128, 2 * bt], DT_MM, tag="xs")
            for m in range(2):
                nc.scalar.activation(
                    xs[:, m * bt:(m + 1) * bt],
                    pr[:, m * bt:(m + 1) * bt],
                    RELU, bias=brsb[:, m:m + 1],
                )

            # heads: py = Wh.T @ xs + bh (bias via ones-row matmul)
            py = psp.tile([8, bt], F32, tag="ph2")
            for k in range(2):
                nc.tensor.matmul(
                    py, whsb[:, k * 8:(k + 1) * 8], xs[:, k * bt:(k + 1) * bt],
                    start=(k == 0), stop=False,
                )
            nc.tensor.matmul(py, bhsb, ones_sb, start=False, stop=True)

            ysb = sbp.tile([8, bt], F32, tag="ysb")
            nc.vector.tensor_scalar(
                ysb, py, clipsb[:, 0:1], clipsb[:, 1:2],
                op0=mybir.AluOpType.min, op1=mybir.AluOpType.max,
            )
            nc.sync.dma_start(out=y_d[:, s0:s0 + bt], in_=ysb)

    return nc


def _get_nc(bc, bt):
    key = (bc, bt)
    if key not in _CACHE:
        nc = _build_bass(bc, bt)
        nc.finalize()  # Bacc: run compile passes (wait-splitting, reg alloc)
        _CACHE[key] = nc
    return _CACHE[key]


def kernel(obs, ag, g, phi_w1, phi_b1, phi_w2, phi_b2,
           rho_w1, rho_b1, mean_w, mean_b, logstd_w, logstd_b):
    obs = np.asarray(obs, np.float32)
    ag = np.asarray(ag, np.float32)
    g = np.asarray(g, np.float32)
    B = obs.shape[0]
    assert B == B_FULL, f"kernel hardcoded for B={B_FULL}, got {B}"

    packed = _pack_weights(phi_w1, phi_b1, phi_w2, phi_b2, rho_w1, rho_b1,
                           mean_w, mean_b, logstd_w, logstd_b)
    xt = _pack_xt(obs, ag, g)

    nc = _get_nc(BC, BT)
    in_maps = []
    for c in range(N_CORES):
        m = dict(packed)
        m["xt"] = np.ascontiguousarray(xt[:, c * BC:(c + 1) * BC])
        in_maps.append(m)

    import os
    trace = bool(os.environ.get("KERNEL_TRACE"))
    res = run_bass_kernel_spmd(nc, in_maps, core_ids=list(range(N_CORES)),
                               trace=trace)
    global _last_results
    _last_results = res

    y = np.concatenate([res.results[c]["y"] for c in range(N_CORES)], axis=1)  # [8, B]
    out = np.ascontiguousarray(y.T)  # [B, 8]
    mean = out[:, 0:4].copy()
    logstd = out[:, 4:8].copy()
    return mean, logstd


_last_results = None
